# revision 45
# baseline (speedup 1.0000x reference)
"""Causal self-attention (B=2, T=2048, C=1024, H=16) on 8 trn2 NeuronCores.

Sharding: tensor-parallel over heads — 2 heads per core. Each core computes
q/k/v projections for its 2 heads (from a replicated transposed x), causal
attention for those heads, and a partial out-projection [B*T, C]; the host
sums the 8 partials and adds the output bias.

Layouts (per core):
  xT    [1024, 4096]  x transposed (c on partitions), host-prepared, bf16
  qT/kT [128, 4096]   head-dim-major (2 heads x 64 dims on partitions)
  v     natural [t, d] per head via PE transpose of vT
  S^T   [k, q] tiles from matmul(lhsT=kT, rhs=qT); softmax runs without the
        max-subtraction (scores are O(few)), the sum over k rides as a
        ones-column in the PV matmul, normalization divides at the end.
"""

import sys

for _p in ("/opt/trn_rl_repo", "/opt/pypackages"):
    if _p not in sys.path:
        sys.path.append(_p)

from contextlib import ExitStack

import numpy as np
import ml_dtypes

import concourse.bass as bass
import concourse.tile as tile
from concourse import bacc, mybir
from concourse.bass import ts, ds
from concourse.bass_utils import run_bass_kernel_spmd
from concourse.masks import make_identity

BF16 = ml_dtypes.bfloat16
F32 = mybir.dt.float32
MBF16 = mybir.dt.bfloat16
AF = mybir.ActivationFunctionType

B, T, C, H = 2, 2048, 1024, 16
HD = C // H              # 64
NCORES = 8
HPC = H // NCORES        # 2 heads per core
BT = B * T               # 4096
SCALE = 1.0 / np.sqrt(HD)
NCO = C // 128           # 8 contraction tiles
NPW = BT // 512          # 8 projection windows
NQW = T // 512           # 4 q-windows per batch
NKT = T // 128           # 16 k-tiles per batch
NTT = BT // 128          # 32 t-tiles


def build_nc(dbg=False):
    nc = bacc.Bacc("TRN2", target_bir_lowering=False, debug=False)

    xT_d = nc.dram_tensor("xT", [NCO, 128, BT], MBF16, kind="ExternalInput").ap()
    wq_d = nc.dram_tensor("wq", [128, NCO, 128], MBF16, kind="ExternalInput").ap()
    wk_d = nc.dram_tensor("wk", [128, NCO, 128], MBF16, kind="ExternalInput").ap()
    wv_d = nc.dram_tensor("wv", [128, NCO, 128], MBF16, kind="ExternalInput").ap()
    bq_d = nc.dram_tensor("bq", [128, 1], F32, kind="ExternalInput").ap()
    bk_d = nc.dram_tensor("bk", [128, 1], F32, kind="ExternalInput").ap()
    bv_d = nc.dram_tensor("bv", [128, 1], F32, kind="ExternalInput").ap()
    wo_d = nc.dram_tensor("wo", [128, C], MBF16, kind="ExternalInput").ap()
    mk_d = nc.dram_tensor("mask", [128, 4, 512], MBF16, kind="ExternalInput").ap()
    out_d = nc.dram_tensor("part", [NTT, 128, C], MBF16, kind="ExternalOutput").ap()

    with tile.TileContext(nc) as tc, ExitStack() as ctx:
        per = ctx.enter_context(tc.tile_pool(name="persist", bufs=1))
        xT = [
            per.tile([128, BT], MBF16, tag=f"xT{i}", name=f"xT{i}") for i in range(NCO)
        ]
        wq = per.tile([128, NCO, 128], MBF16, tag="wq")
        wk = per.tile([128, NCO, 128], MBF16, tag="wk")
        wv = per.tile([128, NCO, 128], MBF16, tag="wv")
        bq = per.tile([128, 1], F32, tag="bq")
        bk = per.tile([128, 1], F32, tag="bk")
        bv = per.tile([128, 1], F32, tag="bv")
        wo = per.tile([128, C], MBF16, tag="wo")
        mk = per.tile([128, 4, 512], MBF16, tag="mk")
        ident = per.tile([128, 128], MBF16, tag="ident")
        qT = per.tile([128, BT], MBF16, tag="qT")
        kT = per.tile([128, BT], MBF16, tag="kT")
        vT = per.tile([128, BT], MBF16, tag="vT")
        # v natural per head, ones-column at col 64 (sumexp rides the PV matmul)
        vA = per.tile([128, NKT * B, 128], MBF16, tag="vA")
        vB = per.tile([128, NKT * B, 128], MBF16, tag="vB")
        # normalized attn out, head-major, one tile per 512-window so the
        # out-projection of window g only depends on window g's writers
        aT = [
            per.tile([128, 512], MBF16, tag=f"aT{g}", name=f"aT{g}")
            for g in range(NPW)
        ]

        # --- input DMAs: small weights first, then x chunks in window-major
        # order (projection window g touches all 8 c-slices of window g), so
        # the first matmuls start after ~1MB instead of ~10MB.
        # weights go on the scalar-engine DMA queue, x on sync's — parallel
        nc.scalar.dma_start(out=wq[:], in_=wq_d)
        nc.scalar.dma_start(out=wk[:], in_=wk_d)
        nc.scalar.dma_start(out=wv[:], in_=wv_d)
        nc.scalar.dma_start(out=bq[:], in_=bq_d)
        nc.scalar.dma_start(out=bk[:], in_=bk_d)
        nc.scalar.dma_start(out=bv[:], in_=bv_d)

        def load_x_window(wi, split=False, width=512):
            for i in range(NCO):
                eng = nc.scalar if (split and i % 2) else nc.sync
                eng.dma_start(
                    out=xT[i][:, wi * 512 : wi * 512 + width],
                    in_=xT_d[i][:, wi * 512 : wi * 512 + width],
                )

        load_x_window(0, split=True)
        nc.scalar.dma_start(out=mk[:], in_=mk_d)
        nc.scalar.dma_start(out=wo[:], in_=wo_d)
        load_x_window(1)

        make_identity(nc, ident[:])
        ones64 = per.tile([1, 64], F32, tag="ones64")
        nc.vector.memset(ones64[:], 1.0)
        nc.vector.memset(vA[:, :, 64:65], 1.0)
        nc.vector.memset(vB[:, :, 64:65], 1.0)

        # Pipelined emission: per 512-wide window g, project q/k/v (window g),
        # PE-transpose v, then attention for window g (its scores only need
        # q/k windows <= g), then the out-projection of window g-1. ACT's exp
        # work overlaps projection matmuls; PE stays dense (HAM stays warm).
        with (
            tc.tile_pool(name="pps", bufs=2, space="PSUM") as pps,
            tc.tile_pool(name="sps", bufs=2, space="PSUM") as sps,
            tc.tile_pool(name="pvp", bufs=2, space="PSUM") as pvp,
            tc.tile_pool(name="eap", bufs=6) as eap,
            tc.tile_pool(name="rp", bufs=4) as rp,
            tc.tile_pool(name="bp", bufs=2) as bp,
            tc.tile_pool(name="tbp", bufs=2) as tbp,
            tc.tile_pool(name="stp", bufs=4) as stp,
        ):

            def proj(w_sb, b_sb, dest, wi):
                ps = pps.tile([128, 512], F32, tag="proj", name="ps")
                for co in range(NCO):
                    nc.tensor.matmul(
                        ps[:],
                        w_sb[:, co, :],
                        xT[co][:, ts(wi, 512)],
                        start=(co == 0),
                        stop=(co == NCO - 1),
                    )
                nc.vector.tensor_scalar_add(dest[:, ts(wi, 512)], ps[:], b_sb[:, 0:1])

            def outproj_one(g, tt, on_act=False):
                a_sl = aT[g][:, ts(tt - 4 * g, 128)]
                op = sps.tile([128, 2, 512], F32, tag="s", name="op")
                nc.tensor.matmul(op[:, 0, :], a_sl, wo[:, 0:512], start=True, stop=True)
                nc.tensor.matmul(
                    op[:, 1, :], a_sl, wo[:, 512:1024], start=True, stop=True
                )
                st = stp.tile([128, 2, 512], MBF16, tag="st")
                if on_act:
                    nc.scalar.activation(st[:], op[:], AF.Copy)
                else:
                    nc.vector.tensor_copy(out=st[:], in_=op[:])
                nc.sync.dma_start(out=out_d[tt], in_=st.rearrange("p a b -> p (a b)"))

            def outproj(g):
                for tt in range(4 * g, 4 * g + 4):
                    outproj_one(g, tt)

            def attention(b, w, filler=None):
                qs = b * T + w * 512
                nk = 4 * (w + 1)
                pva = pvp.tile([128, 512], F32, tag="pv", name="pva")
                pvb = pvp.tile([128, 512], F32, tag="pv", name="pvb")
                def emit_pv(jp, ea, eb):
                    j0 = 2 * jp
                    for (e, vh, pv) in ((ea, vA, pva), (eb, vB, pvb)):
                        for jj, jloc in ((0, j0), (1, j0 + 1)):
                            nc.tensor.matmul(
                                pv[0:65, :],
                                vh[:, b * NKT + jloc, 0:65],
                                e[:, jj, :],
                                start=(jloc == 0),
                                stop=(jloc == nk - 1),
                            )

                pend = None
                for jp in range(nk // 2):
                    j0, j1 = 2 * jp, 2 * jp + 1
                    diag = j0 >= nk - 4
                    # scores: head A (rows 0:64) and head B (rows 64:128)
                    # alternate so the PE overlaps them across row-groups
                    sa = sps.tile([128, 2, 512], F32, tag="s", name="sa")
                    sb_ = sps.tile([128, 2, 512], F32, tag="s", name="sb")
                    for jj, jloc in ((0, j0), (1, j1)):
                        kd = ds(b * T + jloc * 128, 128)
                        nc.tensor.matmul(
                            sa[:, jj, :], kT[0:64, kd], qT[0:64, ds(qs, 512)],
                            start=True, stop=True,
                        )
                        nc.tensor.matmul(
                            sb_[:, jj, :], kT[64:128, kd], qT[64:128, ds(qs, 512)],
                            start=True, stop=True,
                        )
                    es = []
                    for s_ps in (sa, sb_):
                        e = eap.tile([128, 2, 512], MBF16, tag="e")
                        nc.scalar.activation(e[:], s_ps[:], AF.Exp, scale=float(SCALE))
                        if diag:
                            i0 = j0 - (nk - 4)
                            nc.vector.tensor_mul(e[:], e[:], mk[:, i0 : i0 + 2, :])
                        es.append(e)
                    # software pipeline: PV of stage jp-1 issues after scores of
                    # stage jp, so exp latency never blocks the PE stream
                    if pend is not None:
                        emit_pv(*pend)
                    pend = (jp, es[0], es[1])
                # fill the final exp's latency with independent PE work
                if filler is not None:
                    filler()
                emit_pv(*pend)
                return pva, pvb

            def normalize(b, w, pva, pvb, cols=None, pe_bcast=False):
                # rows 0..63 head dims, row 64 sumexp
                g = NQW * b + w
                c0, cn = cols if cols else (0, 512)
                cs = ds(c0, cn)
                for (pv, hlo) in ((pva, 0), (pvb, 64)):
                    # custom-DVE recip misreads PSUM on HW: copy to SBUF first
                    sm = rp.tile([1, 512], F32, tag="sm", name="sm")
                    nc.scalar.activation(sm[0:1, 0:cn], pv[64:65, cs], AF.Copy)
                    rc = rp.tile([1, 512], F32, tag="rc", name="rc")
                    nc.vector.reciprocal_approx_fast(out=rc[0:1, 0:cn], in_=sm[0:1, 0:cn])
                    bc = bp.tile([64, 512], F32, tag="bc", name="bc")
                    nc.gpsimd.partition_broadcast(bc[:, 0:cn], rc[0:1, 0:cn], channels=64)
                    if hlo == 0:
                        nc.vector.tensor_mul(aT[g][0:64, cs], pv[0:64, cs], bc[:, 0:cn])
                    else:
                        tb = tbp.tile([64, 512], MBF16, tag="tb")
                        nc.vector.tensor_mul(tb[:, 0:cn], pv[0:64, cs], bc[:, 0:cn])
                        # head B lives on partitions 64..127 of aT; DVE can't
                        # cross partitions, so hop through an SBUF->SBUF DMA.
                        nc.sync.dma_start(out=aT[g][64:128, cs], in_=tb[:, 0:cn])

            for g in range(NPW):
                if g % 2 == 0 and g + 2 < NPW:
                    load_x_window(g + 2, width=1024)
                proj(wq, bq, qT, g)
                proj(wk, bk, kT, g)
                proj(wv, bv, vT, g)
                for j in range(4 * g, 4 * g + 4):
                    tp = pps.tile([128, 128], MBF16, tag="proj", name="tp")
                    nc.tensor.transpose(tp[:], vT[:, ts(j, 128)], ident[:])
                    nc.vector.tensor_copy(out=vA[:, j, 0:64], in_=tp[:, 0:64])
                    nc.vector.tensor_copy(out=vB[:, j, 0:64], in_=tp[:, 64:128])
                # out-projection of the previous window fills the final exp
                # latency inside attention; its PSUM->SBUF copies also reach
                # the DVE queue ahead of this window's normalize chain
                filler = (lambda gp=g - 1: outproj(gp)) if g >= 1 else None
                pva, pvb = attention(g // NQW, g % NQW, filler=filler)
                if g < NPW - 1:
                    normalize(g // NQW, g % NQW, pva, pvb)
            # last window: normalize/out-project in half-window chunks so the
            # final out-proj matmuls overlap the second half's normalize
            glast = NPW - 1
            normalize(B - 1, NQW - 1, pva, pvb, cols=(0, 256), pe_bcast=True)
            outproj_tiles = list(range(4 * glast, 4 * glast + 4))
            for tt in outproj_tiles[:2]:
                outproj_one(glast, tt, on_act=(tt % 2 == 0))
            normalize(B - 1, NQW - 1, pva, pvb, cols=(256, 256), pe_bcast=True)
            for tt in outproj_tiles[2:]:
                outproj_one(glast, tt, on_act=(tt % 2 == 0))

        if dbg:
            for name, t in (("qTd", qT), ("kTd", kT), ("vTd", vT)):
                d = nc.dram_tensor(name, [128, BT], MBF16, kind="ExternalOutput").ap()
                nc.sync.dma_start(out=d, in_=t[:])
            aTd = nc.dram_tensor("aTd", [128, BT], MBF16, kind="ExternalOutput").ap()
            for g in range(NPW):
                nc.sync.dma_start(out=aTd[:, ts(g, 512)], in_=aT[g][:])
            for name, t in (("vAd", vA), ("vBd", vB)):
                d = nc.dram_tensor(
                    name, [128, NKT * B, 65], MBF16, kind="ExternalOutput"
                ).ap()
                nc.sync.dma_start(out=d, in_=t[:, :, 0:65])

    nc.compile()
    return nc


_NC = None


def _get_nc():
    global _NC
    if _NC is None:
        _NC = build_nc()
    return _NC


def _make_in_maps(x, w_qkv, b_qkv, w_out):
    xT = np.ascontiguousarray(x.reshape(BT, C).T).astype(BF16).reshape(NCO, 128, BT)
    p = np.arange(128)[:, None]
    f = np.arange(512)[None, :]
    mask = np.stack([(128 * i + p <= f) for i in range(4)], axis=1).astype(BF16)
    in_maps = []
    for i in range(NCORES):
        sl = slice(128 * i, 128 * i + 128)
        m = {
            "xT": xT,
            "wq": np.ascontiguousarray(
                w_qkv[:, sl].reshape(NCO, 128, 128).transpose(1, 0, 2)
            ).astype(BF16),
            "wk": np.ascontiguousarray(
                w_qkv[:, C + 128 * i : C + 128 * i + 128]
                .reshape(NCO, 128, 128)
                .transpose(1, 0, 2)
            ).astype(BF16),
            "wv": np.ascontiguousarray(
                w_qkv[:, 2 * C + 128 * i : 2 * C + 128 * i + 128]
                .reshape(NCO, 128, 128)
                .transpose(1, 0, 2)
            ).astype(BF16),
            "bq": b_qkv[sl].astype(np.float32).reshape(128, 1),
            "bk": b_qkv[C + 128 * i : C + 128 * i + 128].astype(np.float32).reshape(128, 1),
            "bv": b_qkv[2 * C + 128 * i : 2 * C + 128 * i + 128]
            .astype(np.float32)
            .reshape(128, 1),
            "wo": np.ascontiguousarray(w_out[sl, :]).astype(BF16),
            "mask": mask,
        }
        in_maps.append(m)
    return in_maps


def run(inputs, trace=False):
    """Returns (y, exec_time_ns_or_None)."""
    x = np.asarray(inputs["x"], dtype=np.float32)
    w_qkv = np.asarray(inputs["w_qkv"], dtype=np.float32)
    b_qkv = np.asarray(inputs["b_qkv"], dtype=np.float32)
    w_out = np.asarray(inputs["w_out"], dtype=np.float32)
    b_out = np.asarray(inputs["b_out"], dtype=np.float32)

    nc = _get_nc()
    in_maps = _make_in_maps(x, w_qkv, b_qkv, w_out)
    res = run_bass_kernel_spmd(nc, in_maps, list(range(NCORES)), trace=trace)
    part = np.zeros((NTT, 128, C), dtype=np.float32)
    for r in res.results:
        part += r["part"]
    y = part.reshape(BT, C) + b_out[None, :]
    return y.reshape(B, T, C).astype(np.float32), res.exec_time_ns


def kernel(**inputs):
    return run(inputs, trace=False)[0]
